# revision 22
# baseline (speedup 1.0000x reference)
"""Trainium2 Bass kernel: causal GQA attention.

Problem: B=2, Sq=Sk=2048, H=32, Hkv=8, D=128, fp32, causal + key-padding mask.

Sharding (8 cores): head-parallel. Core c takes q-heads [4c, 4c+4) for both
batches; those 4 heads share exactly one kv head (c) per batch, so each core
runs 8 independent (batch, head) pairs — K/V loaded once per batch, no comms.

All matmuls run in bf16 (1 PE cycle/row at ANY output width — unlike fp32r's
>=256 constraint — so diagonal chunks are sliced to exactly their live
columns). Scores are built TRANSPOSED (keys on partitions, queries on free) so
softmax-weight x V contracts the key axis directly with V in natural layout.

Work is organized as a flat, software-pipelined stream of "batches" (1-3 key
chunks sharing one score tile and one exp instruction) spanning all pairs and
query groups: batch i+1's QK matmuls are emitted BEFORE batch i's PV matmuls
so the PE always has independent work while the ACT engine (the bottleneck)
runs exp back-to-back.

Per (pair, 512-query group g): off-diagonal chunks in batches of 3; one
packed diagonal batch (4 chunks column-packed at offsets {0,512,896,1024} so
no matmul output crosses a PSUM bank and one exp covers all 1280 live cols):
    S^T = K_j @ Q_g^T            (PE bf16 -> PSUM fp32, diag sliced to live cols)
    P^T = exp(scale*S^T)         (ACT, one instr per batch, PSUM->SBUF bf16)
    diag triangles: P^T *= tri01 (DVE bf16, [128,128] per diag chunk)
    O/sums fused:  acc[t] += P^T[:, t-tile].T @ [V_j | 1]   (PE, 129-wide
                   moving operand: the appended ones column accumulates the
                   softmax denominator in the same matmul)
  acc layout: 2 PSUM banks per group, each holding 2 q-tiles [128, 129].
  One start=True per bank (zeroes the whole 2KB zero region) and one
  stop=True per bank; in-between matmuls accumulate with start=False.
  rsum = 1/acc[:, :, 128]        (DVE reciprocal)
  out  = acc[:, t, 0:128] * rsum[t]   (DVE tensor_scalar per-partition, ->bf16)
  DMA out [128q, 4t, 128d]; host reassembles (no transposes on device).

Two off-diag batches per pair (SCH_BATCHES) skip ACT entirely: with Q
pre-scaled by A_SCH on the host, exp becomes a single DVE tensor_scalar
(add B_SCH, convert f32->i16, bitcast bf16) - Schraudolph's approximation
in bf16, validated to ~9e-3 rel err even if applied everywhere.

PSUM: 2 x 3-bank rotating score tiles + 2 single-buffered accumulator banks
= all 8 banks. Cost-model engine busy: PE 118us, ACT 115us, DVE 86us of
146us total (baseline was 197us).
"""

import math
import sys

import numpy as np

for _p in ("/opt/trn_rl_repo",):
    if _p not in sys.path:
        sys.path.append(_p)

import concourse.bass as bass
import concourse.tile as tile
from concourse import bacc, mybir
from concourse.bass import ts
from concourse.bass_utils import run_bass_kernel_spmd

B = 2
S = 2048
H = 32
HKV = 8
D = 128
N_CORES = 8
HPC = H // N_CORES  # q heads per core = 4
PAIRS = B * HPC  # 8 (batch, head) pairs per core
NG = S // 512  # 4 q-groups of 512 per pair
NCHUNK = S // 128  # 16 key chunks of 128
VW = 132  # padded vx row: 128 v cols + 1 ones col + 3 pad
SCALE = 1.0 / math.sqrt(D)
NEG = -10000.0
# Schraudolph-in-bf16 exp for DVE-offloaded batches: host pre-scales Q by
# A_SCH so st = A_SCH * (q.k); then bf16(exp(scale*q.k)) ~= bitcast_i16(
# st + B_SCH) and the exact-exp ACT batches just use scale SCALE_ACT.
A_SCH = SCALE * 128.0 * math.log2(math.e)
B_SCH = 127.0 * 128.0 - 486411.0 / 65536.0 + 0.5  # +0.5: f32->i16 truncates
SCALE_ACT = SCALE / A_SCH  # = ln2/128
# off-diag 3-chunk batches per (g, index-within-g) offloaded to DVE exp
SCH_BATCHES = {(3, 0), (3, 2)}

# column offset of diagonal chunk u inside the packed [128, 1280] score tile;
# widths are 512-128u, arranged so no matmul output crosses a 2KB PSUM bank:
# bank0: u0 (512 cols); bank1: u1 (384) + u3 (128); bank2: u2 (256)
DIAG_OFF = (0, 512, 1024, 896)
DIAG_COLS = 1280

F32 = mybir.dt.float32
BF16 = mybir.dt.bfloat16
EXP = mybir.ActivationFunctionType.Exp


class _Batch:
    __slots__ = (
        "pair", "b", "g", "chunks", "diag", "first_of_group", "last_of_group",
        "first_of_pair", "st", "pt", "grp", "sch",
    )

    def __init__(self, pair, b, g, chunks, diag, first_of_group, last_of_group):
        self.sch = False
        self.pair = pair
        self.b = b
        self.g = g
        self.chunks = chunks  # list of key-chunk indices j
        self.diag = diag
        self.first_of_group = first_of_group
        self.last_of_group = last_of_group
        self.st = None
        self.pt = None
        self.grp = None  # (acc0, acc1) for this (pair, g)


def build_module(uniform_mask: bool = True):
    nc = bacc.Bacc("TRN2", target_bir_lowering=False, debug=False, num_devices=1)

    qt = nc.dram_tensor("qt", [PAIRS, D, S], BF16, kind="ExternalInput").ap()
    kt = nc.dram_tensor("kt", [B, D, S], BF16, kind="ExternalInput").ap()
    vx = nc.dram_tensor("vx", [B, D, NCHUNK, VW], BF16, kind="ExternalInput").ap()
    tri = nc.dram_tensor("tri", [D, D], BF16, kind="ExternalInput").ap()
    pb = nc.dram_tensor("pb", [B, S], F32, kind="ExternalInput").ap()
    ot = nc.dram_tensor("ot", [PAIRS, NG, D, 4, D], BF16, kind="ExternalOutput").ap()

    # flat batch schedule for the whole core
    sched = []
    for b in range(B):
        for h in range(HPC):
            pair = b * HPC + h
            for g in range(NG):
                nfull = 4 * g
                # balanced batch sizes (max 3 chunks), smallest first: a
                # short exp window right before the diag handoff compounds PE
                # overflow at the least elastic point; early, its overflow is
                # absorbed by later windows' slack
                groups = []
                if nfull:
                    nb_ = -(-nfull // 3)
                    base, rem = divmod(nfull, nb_)
                    sizes = [base] * (nb_ - rem) + [base + 1] * rem
                    s0 = 0
                    for sz in sizes:
                        groups.append(list(range(s0, s0 + sz)))
                        s0 += sz
                for bi, ch in enumerate(groups):
                    nb = _Batch(pair, b, g, ch, False, bi == 0, False)
                    nb.sch = (
                        uniform_mask
                        and (g, bi) in SCH_BATCHES
                        and len(ch) == 3
                        # the final pair's last sch batch would land in the
                        # drain tail where ACT idle is free anyway
                        and not (pair == PAIRS - 1 and (g, bi) == (3, 3))
                    )
                    sched.append(nb)
                sched.append(
                    _Batch(
                        pair, b, g, [4 * g + u for u in range(4)], True,
                        len(groups) == 0, True,
                    )
                )
    first_idx = {}  # pair -> index of its first batch
    for i, bb in enumerate(sched):
        if bb.pair not in first_idx:
            first_idx[bb.pair] = i

    with tile.TileContext(nc) as tc:
        with (
            tc.tile_pool(name="consts", bufs=1) as consts,
            tc.tile_pool(name="kv", bufs=2) as kv_pool,
            tc.tile_pool(name="q", bufs=3) as q_pool,
            tc.tile_pool(name="pt", bufs=4) as pt_pool,
            tc.tile_pool(name="osb", bufs=3) as osb_pool,
            tc.tile_pool(name="small", bufs=4) as small_pool,
            tc.tile_pool(name="st_ps", bufs=2, space="PSUM") as st_pool,
            tc.tile_pool(name="acc_ps", bufs=2, space="PSUM") as acc_pool,
        ):
            tri_sb = consts.tile([D, D], BF16)
            nc.sync.dma_start(tri_sb[:], tri[:])
            # warm the ACT exp table during the initial DMAs
            warm_i = consts.tile([1, 2], F32)
            warm_o = consts.tile([1, 2], F32)
            nc.vector.memset(warm_i[:], 0.0)
            nc.scalar.activation(warm_o[:], warm_i[:], EXP, scale=1.0)

            kt_sbs, vx_sbs, pb_sbs, qt_sbs = {}, {}, {}, {}

            def _load_kv(b):
                # split loads so the first group's compute starts after the
                # first slices. At kernel start (b=0) the DMA generation
                # latency is on the critical path, so spread the three
                # streams across all three DGE queues (sync/scalar/gpsimd);
                # mid-stream (b=1) keep everything off the ACT sequencer.
                kt_sb = kv_pool.tile([D, S], BF16, tag="kt")
                vx_sb = kv_pool.tile([D, NCHUNK, VW], BF16, tag="vx")
                qt0_sb = q_pool.tile([D, S], BF16, tag="qt")
                for q4 in range(4):
                    nc.sync.dma_start(kt_sb[:, ts(q4, 512)], kt[b][:, ts(q4, 512)])
                    nc.sync.dma_start(
                        qt0_sb[:, ts(q4, 512)], qt[b * HPC][:, ts(q4, 512)]
                    )
                    nc.sync.dma_start(vx_sb[:, ts(q4, 4), :], vx[b][:, ts(q4, 4), :])
                pb_sb = kv_pool.tile([D, NCHUNK], F32, tag="pb")
                nc.gpsimd.dma_start(pb_sb[:], pb[b].rearrange("(j k) -> k j", k=128))
                kt_sbs[b], vx_sbs[b], pb_sbs[b] = kt_sb, vx_sb, pb_sb
                qt_sbs[b * HPC] = qt0_sb

            def _load_qt(pair):
                qt_sb = q_pool.tile([D, S], BF16, tag="qt")
                for q4 in range(4):
                    nc.sync.dma_start(
                        qt_sb[:, ts(q4, 512)], qt[pair][:, ts(q4, 512)]
                    )
                qt_sbs[pair] = qt_sb

            def emit_qk(bb):
                bb.st = st_pool.tile([D, 1536], F32, tag="st")
                qt_sb = qt_sbs[bb.pair]
                kt_sb = kt_sbs[bb.b]
                if bb.diag:
                    # modest priority boost: at equal readiness the scheduler
                    # must pick these over backlogged PVs or the diag exp
                    # (and the whole group handoff) slips by ~1us
                    with tc.high_priority(offset=40):
                        for u in range(4):
                            j = 4 * bb.g + u
                            off = DIAG_OFF[u]
                            w = 512 - 128 * u
                            nc.tensor.matmul(
                                bb.st[:, off : off + w],
                                lhsT=kt_sb[:, ts(j, 128)],
                                rhs=qt_sb[:, 512 * bb.g + 128 * u : 512 * (bb.g + 1)],
                                start=True,
                                stop=True,
                            )
                else:
                    for i, j in enumerate(bb.chunks):
                        nc.tensor.matmul(
                            bb.st[:, ts(i, 512)],
                            lhsT=kt_sb[:, ts(j, 128)],
                            rhs=qt_sb[:, ts(bb.g, 512)],
                            start=True,
                            stop=True,
                        )

            def emit_exp(bb):
                bb.pt = pt_pool.tile([D, 1536], BF16, tag="pt")
                pb_sb = pb_sbs[bb.b]
                if bb.diag:
                    if uniform_mask:
                        nc.scalar.activation(
                            bb.pt[:, :DIAG_COLS], bb.st[:, :DIAG_COLS], EXP,
                            scale=SCALE_ACT,
                        )
                    else:
                        for u in range(4):
                            j = 4 * bb.g + u
                            off = DIAG_OFF[u]
                            w = 512 - 128 * u
                            nc.scalar.activation(
                                bb.pt[:, off : off + w], bb.st[:, off : off + w],
                                EXP, bias=pb_sb[:, j : j + 1], scale=SCALE_ACT,
                            )
                    # zero the upper triangle of each diag chunk's first live
                    # 128-col slice (its t=u tile): P^T *= tri01
                    for u in range(4):
                        off = DIAG_OFF[u]
                        nc.vector.tensor_tensor(
                            bb.pt[:, off : off + 128],
                            bb.pt[:, off : off + 128],
                            tri_sb[:],
                            mybir.AluOpType.mult,
                        )
                elif bb.sch:
                    # Schraudolph exp on DVE: one add + f32->i16 convert;
                    # the i16 bits ARE the bf16 probabilities
                    with nc.allow_low_precision(
                        reason="approximate exp within validated tolerance"
                    ):
                        nc.vector.tensor_scalar(
                            bb.pt[:, :1536].bitcast(mybir.dt.int16),
                            bb.st[:, :1536],
                            float(B_SCH),
                            None,
                            mybir.AluOpType.add,
                        )
                else:
                    L = len(bb.chunks)
                    if uniform_mask:
                        nc.scalar.activation(
                            bb.pt[:, : 512 * L], bb.st[:, : 512 * L], EXP,
                            scale=SCALE_ACT,
                        )
                    else:
                        for i, j in enumerate(bb.chunks):
                            nc.scalar.activation(
                                bb.pt[:, ts(i, 512)], bb.st[:, ts(i, 512)], EXP,
                                bias=pb_sb[:, j : j + 1], scale=SCALE,
                            )

            def emit_pv(bb):
                if bb.first_of_group:
                    # each acc tile is exactly one PSUM bank holding 2 q-tiles
                    # of the group; [:, t, 128] is the softmax denominator
                    acc0 = acc_pool.tile([D, 2, 256], F32, tag="acc")
                    acc1 = acc_pool.tile([D, 2, 256], F32, tag="acc")
                    bb.grp = (acc0, acc1)
                    grp_accs[(bb.pair, bb.g)] = bb.grp
                acc0, acc1 = grp_accs[(bb.pair, bb.g)]
                vx_sb = vx_sbs[bb.b]
                g = bb.g
                for ci, j in enumerate(bb.chunks):
                    if bb.diag:
                        u = j - 4 * g
                        off = DIAG_OFF[u]
                        t_lo = u
                    else:
                        off = 512 * ci
                        t_lo = 0
                    first_j = bb.first_of_group and ci == 0
                    for t in range(t_lo, 4):
                        acc = acc0 if t < 2 else acc1
                        tsub = t % 2
                        if bb.diag:
                            pt_ap = bb.pt[
                                :, off + 128 * (t - t_lo) : off + 128 * (t - t_lo) + 128
                            ]
                        else:
                            pt_ap = bb.pt[:, off + 128 * t : off + 128 * t + 128]
                        # one start per bank (zeroes the whole 2KB zero
                        # region); one stop per bank on its last contributor:
                        # bank0's last is diag u1 (j=4g+1) t=1, bank1's last
                        # is diag u3 (j=4g+3) t=3.
                        st_flag = first_j and t in (0, 2)
                        if t < 2:
                            sp_flag = (j == 4 * g + 1) and t == 1
                        else:
                            sp_flag = (j == 4 * g + 3) and t == 3
                        nc.tensor.matmul(
                            acc[:, tsub, 0:129],
                            lhsT=pt_ap,
                            rhs=vx_sb[:, j, 0:129],
                            start=st_flag,
                            stop=sp_flag,
                        )

            def emit_norm(bb):
                acc0, acc1 = grp_accs.pop((bb.pair, bb.g))
                # copy PSUM accumulators out immediately (frees each bank for
                # the next group's start=True zeroing ~400ns after last PV);
                # recip+normalize then run off the critical path from SBUF
                oc = osb_pool.tile([D, 4, 129], F32, tag="oc")
                nc.vector.tensor_copy(oc[:, 0:2, :], acc0[:, :, 0:129])
                nc.vector.tensor_copy(oc[:, 2:4, :], acc1[:, :, 0:129])
                osb = osb_pool.tile([D, 4, D], BF16, tag="osb")
                rsum = small_pool.tile([D, 4, 1], F32, tag="rs")
                nc.vector.reciprocal(rsum[:], oc[:, :, 128:129])
                with nc.allow_low_precision(
                    reason="bf16 output rounding is within tolerance"
                ):
                    for t in range(4):
                        nc.vector.tensor_scalar(
                            osb[:, t, :],
                            oc[:, t, 0:128],
                            rsum[:, t, :],
                            None,
                            mybir.AluOpType.mult,
                        )
                nc.sync.dma_start(ot[bb.pair, bb.g], osb[:])

            grp_accs = {}
            _load_kv(0)

            # prologue actions keyed by emission slot: qt prefetches for
            # same-batch pairs ~6 batches early; batch-1 KV load during pair 2
            prologue = {}
            for pair in range(1, PAIRS):
                if pair == HPC:
                    continue  # loaded by _load_kv(1)
                prologue.setdefault(max(0, first_idx[pair] - 6), []).append(
                    lambda p=pair: _load_qt(p)
                )
            prologue.setdefault(first_idx[3], []).append(lambda: _load_kv(1))

            # 2-deep software pipeline: in the window where ACT runs exp(k),
            # the PE executes QK(k+1) first (data ready at window start, so
            # exp(k+1) is never gated by QK latency) and then PV(k-1) (whose
            # pt has been ready since exp(k-1) ended - no sem-latency stall).
            n = len(sched)
            for i in range(n + 2):
                for fn in prologue.get(i, []):
                    fn()
                if i < n:
                    emit_qk(sched[i])
                if 1 <= i <= n:
                    emit_exp(sched[i - 1])
                if 2 <= i:
                    bb = sched[i - 2]
                    emit_pv(bb)
                    if bb.last_of_group:
                        emit_norm(bb)

    nc.compile()
    return nc


_NC = {}


def _get_nc(uniform_mask: bool = True):
    if uniform_mask not in _NC:
        _NC[uniform_mask] = build_module(uniform_mask)
    return _NC[uniform_mask]


def shard_inputs(q, kv, key_padding_mask):
    """Full inputs -> list of 8 per-core input maps."""
    import ml_dtypes

    bf16 = ml_dtypes.bfloat16
    q = np.asarray(q, dtype=np.float32)
    kv = np.asarray(kv, dtype=np.float32)
    mask = np.asarray(key_padding_mask)

    pbias = np.where(mask, np.float32(0.0), np.float32(NEG)).astype(np.float32)

    # tri01[k, qq] = 1 where k <= qq (keys on partitions)
    kk = np.arange(128)[:, None]
    qq = np.arange(128)[None, :]
    tri01 = (kk <= qq).astype(bf16)

    in_maps = []
    for c in range(N_CORES):
        qc = q[:, :, HPC * c : HPC * (c + 1), :]  # [B, S, 4, D]
        qtc = (
            (np.ascontiguousarray(np.transpose(qc, (0, 2, 3, 1)))
             * np.float32(A_SCH))
            .reshape(PAIRS, D, S)
            .astype(bf16)
        )
        kc = kv[:, :, 0, c, :]  # [B, S, D]
        vc = kv[:, :, 1, c, :]  # [B, S, D]
        ktc = np.ascontiguousarray(np.transpose(kc, (0, 2, 1))).astype(bf16)
        # vx[b, k, j, 0:128] = v[b, 128j+k, :]; [..., 128] = 1; pad 0
        vxc = np.zeros((B, D, NCHUNK, VW), dtype=bf16)
        vxc[:, :, :, :128] = np.transpose(
            vc.reshape(B, NCHUNK, 128, D), (0, 2, 1, 3)
        )
        vxc[:, :, :, 128] = bf16(1.0)
        in_maps.append(
            {"qt": qtc, "kt": ktc, "vx": vxc, "tri": tri01, "pb": pbias}
        )
    return in_maps


def unshard_output(results):
    """Per-core 'ot' [PAIRS, NG, 128, 4, 128] -> full [B, S, H, D] fp32."""
    out = np.empty((B, S, H, D), dtype=np.float32)
    for c in range(N_CORES):
        otc = np.asarray(results[c]["ot"], dtype=np.float32)
        for pair in range(PAIRS):
            b, h = pair // HPC, HPC * c + pair % HPC
            # [NG, 128p, 4t, D] -> [NG, 4t, 128p, D] -> [S, D]
            out[b, :, h, :] = np.transpose(otc[pair], (0, 2, 1, 3)).reshape(S, D)
    return out


def kernel(q, kv, key_padding_mask):
    uniform = bool(np.asarray(key_padding_mask).all())
    nc = _get_nc(uniform)
    in_maps = shard_inputs(q, kv, key_padding_mask)
    res = run_bass_kernel_spmd(nc, in_maps, core_ids=list(range(N_CORES)))
    return unshard_output(res.results)


# revision 24
# speedup vs baseline: 1.0037x; 1.0037x over previous
"""Trainium2 Bass kernel: causal GQA attention.

Problem: B=2, Sq=Sk=2048, H=32, Hkv=8, D=128, fp32, causal + key-padding mask.

Sharding (8 cores): head-parallel. Core c takes q-heads [4c, 4c+4) for both
batches; those 4 heads share exactly one kv head (c) per batch, so each core
runs 8 independent (batch, head) pairs — K/V loaded once per batch, no comms.

All matmuls run in bf16 (1 PE cycle/row at ANY output width — unlike fp32r's
>=256 constraint — so diagonal chunks are sliced to exactly their live
columns). Scores are built TRANSPOSED (keys on partitions, queries on free) so
softmax-weight x V contracts the key axis directly with V in natural layout.

Work is organized as a flat, software-pipelined stream of "batches" (1-3 key
chunks sharing one score tile and one exp instruction) spanning all pairs and
query groups: batch i+1's QK matmuls are emitted BEFORE batch i's PV matmuls
so the PE always has independent work while the ACT engine (the bottleneck)
runs exp back-to-back.

Per (pair, 512-query group g): off-diagonal chunks in batches of 3; one
packed diagonal batch (4 chunks column-packed at offsets {0,512,896,1024} so
no matmul output crosses a PSUM bank and one exp covers all 1280 live cols):
    S^T = K_j @ Q_g^T            (PE bf16 -> PSUM fp32, diag sliced to live cols)
    P^T = exp(scale*S^T)         (ACT, one instr per batch, PSUM->SBUF bf16)
    diag triangles: P^T *= tri01 (DVE bf16, [128,128] per diag chunk)
    O/sums fused:  acc[t] += P^T[:, t-tile].T @ [V_j | 1]   (PE, 129-wide
                   moving operand: the appended ones column accumulates the
                   softmax denominator in the same matmul)
  acc layout: 2 PSUM banks per group, each holding 2 q-tiles [128, 129].
  One start=True per bank (zeroes the whole 2KB zero region) and one
  stop=True per bank; in-between matmuls accumulate with start=False.
  rsum = 1/acc[:, :, 128]        (DVE reciprocal)
  out  = acc[:, t, 0:128] * rsum[t]   (DVE tensor_scalar per-partition, ->bf16)
  DMA out [128q, 4t, 128d]; host reassembles (no transposes on device).

Two off-diag batches per pair (SCH_BATCHES) skip ACT entirely: with Q
pre-scaled by A_SCH on the host, exp becomes a single DVE tensor_scalar
(add B_SCH, convert f32->i16, bitcast bf16) - Schraudolph's approximation
in bf16, validated to ~9e-3 rel err even if applied everywhere.

PSUM: 2 x 3-bank rotating score tiles + 2 single-buffered accumulator banks
= all 8 banks. Cost-model engine busy: PE 118us, ACT 115us, DVE 86us of
146us total (baseline was 197us).
"""

import math
import sys

import numpy as np

for _p in ("/opt/trn_rl_repo",):
    if _p not in sys.path:
        sys.path.append(_p)

import concourse.bass as bass
import concourse.tile as tile
from concourse import bacc, mybir
from concourse.bass import ts
from concourse.bass_utils import run_bass_kernel_spmd

B = 2
S = 2048
H = 32
HKV = 8
D = 128
N_CORES = 8
HPC = H // N_CORES  # q heads per core = 4
PAIRS = B * HPC  # 8 (batch, head) pairs per core
NG = S // 512  # 4 q-groups of 512 per pair
NCHUNK = S // 128  # 16 key chunks of 128
VW = 132  # padded vx row: 128 v cols + 1 ones col + 3 pad
SCALE = 1.0 / math.sqrt(D)
NEG = -10000.0
# Schraudolph-in-bf16 exp for DVE-offloaded batches: host pre-scales Q by
# A_SCH so st = A_SCH * (q.k); then bf16(exp(scale*q.k)) ~= bitcast_i16(
# st + B_SCH) and the exact-exp ACT batches just use scale SCALE_ACT.
A_SCH = SCALE * 128.0 * math.log2(math.e)
B_SCH = 127.0 * 128.0 - 486411.0 / 65536.0 + 0.5  # +0.5: f32->i16 truncates
SCALE_ACT = SCALE / A_SCH  # = ln2/128
# off-diag 3-chunk batches per (g, index-within-g) offloaded to DVE exp
SCH_BATCHES = {(3, 1)}

# column offset of diagonal chunk u inside the packed [128, 1280] score tile;
# widths are 512-128u, arranged so no matmul output crosses a 2KB PSUM bank:
# bank0: u0 (512 cols); bank1: u1 (384) + u3 (128); bank2: u2 (256)
DIAG_OFF = (0, 512, 1024, 896)
DIAG_COLS = 1280

F32 = mybir.dt.float32
BF16 = mybir.dt.bfloat16
EXP = mybir.ActivationFunctionType.Exp


class _Batch:
    __slots__ = (
        "pair", "b", "g", "chunks", "diag", "first_of_group", "last_of_group",
        "first_of_pair", "st", "pt", "grp", "sch",
    )

    def __init__(self, pair, b, g, chunks, diag, first_of_group, last_of_group):
        self.sch = False
        self.pair = pair
        self.b = b
        self.g = g
        self.chunks = chunks  # list of key-chunk indices j
        self.diag = diag
        self.first_of_group = first_of_group
        self.last_of_group = last_of_group
        self.st = None
        self.pt = None
        self.grp = None  # (acc0, acc1) for this (pair, g)


def build_module(uniform_mask: bool = True):
    nc = bacc.Bacc("TRN2", target_bir_lowering=False, debug=False, num_devices=1)

    qt = nc.dram_tensor("qt", [PAIRS, D, S], BF16, kind="ExternalInput").ap()
    kt = nc.dram_tensor("kt", [B, D, S], BF16, kind="ExternalInput").ap()
    vx = nc.dram_tensor("vx", [B, D, NCHUNK, VW], BF16, kind="ExternalInput").ap()
    tri = nc.dram_tensor("tri", [D, D], BF16, kind="ExternalInput").ap()
    pb = nc.dram_tensor("pb", [B, S], F32, kind="ExternalInput").ap()
    ot = nc.dram_tensor("ot", [PAIRS, NG, D, 4, D], BF16, kind="ExternalOutput").ap()

    # flat batch schedule for the whole core
    sched = []
    for b in range(B):
        for h in range(HPC):
            pair = b * HPC + h
            # the final pair runs its groups largest-first: the pipeline
            # drain after the last exp then only carries g0's small diag
            # PV + normalize instead of g3's full backlog
            g_order = range(NG - 1, -1, -1) if pair == PAIRS - 1 else range(NG)
            for g in g_order:
                nfull = 4 * g
                # balanced batch sizes (max 3 chunks), smallest first: a
                # short exp window right before the diag handoff compounds PE
                # overflow at the least elastic point; early, its overflow is
                # absorbed by later windows' slack
                groups = []
                if nfull:
                    nb_ = -(-nfull // 3)
                    base, rem = divmod(nfull, nb_)
                    sizes = [base] * (nb_ - rem) + [base + 1] * rem
                    s0 = 0
                    for sz in sizes:
                        groups.append(list(range(s0, s0 + sz)))
                        s0 += sz
                for bi, ch in enumerate(groups):
                    nb = _Batch(pair, b, g, ch, False, bi == 0, False)
                    nb.sch = (
                        uniform_mask
                        and (g, bi) in SCH_BATCHES
                        and len(ch) == 3
                        # the final pair's last sch batch would land in the
                        # drain tail where ACT idle is free anyway
                        and not (pair == PAIRS - 1 and (g, bi) == (3, 3))
                    )
                    sched.append(nb)
                sched.append(
                    _Batch(
                        pair, b, g, [4 * g + u for u in range(4)], True,
                        len(groups) == 0, True,
                    )
                )
    first_idx = {}  # pair -> index of its first batch
    for i, bb in enumerate(sched):
        if bb.pair not in first_idx:
            first_idx[bb.pair] = i

    with tile.TileContext(nc) as tc:
        with (
            tc.tile_pool(name="consts", bufs=1) as consts,
            tc.tile_pool(name="kv", bufs=2) as kv_pool,
            tc.tile_pool(name="q", bufs=3) as q_pool,
            tc.tile_pool(name="pt", bufs=4) as pt_pool,
            tc.tile_pool(name="osb", bufs=3) as osb_pool,
            tc.tile_pool(name="small", bufs=4) as small_pool,
            tc.tile_pool(name="st_ps", bufs=2, space="PSUM") as st_pool,
            tc.tile_pool(name="acc_ps", bufs=2, space="PSUM") as acc_pool,
        ):
            tri_sb = consts.tile([D, D], BF16)
            nc.sync.dma_start(tri_sb[:], tri[:])
            # warm the ACT exp table during the initial DMAs
            warm_i = consts.tile([1, 2], F32)
            warm_o = consts.tile([1, 2], F32)
            nc.vector.memset(warm_i[:], 0.0)
            nc.scalar.activation(warm_o[:], warm_i[:], EXP, scale=1.0)

            kt_sbs, vx_sbs, pb_sbs, qt_sbs = {}, {}, {}, {}

            def _load_kv(b):
                # split loads so the first group's compute starts after the
                # first slices. At kernel start (b=0) the DMA generation
                # latency is on the critical path, so spread the three
                # streams across all three DGE queues (sync/scalar/gpsimd);
                # mid-stream (b=1) keep everything off the ACT sequencer.
                kt_sb = kv_pool.tile([D, S], BF16, tag="kt")
                vx_sb = kv_pool.tile([D, NCHUNK, VW], BF16, tag="vx")
                qt0_sb = q_pool.tile([D, S], BF16, tag="qt")
                def _kt_qt(q4):
                    nc.sync.dma_start(kt_sb[:, ts(q4, 512)], kt[b][:, ts(q4, 512)])
                    nc.sync.dma_start(
                        qt0_sb[:, ts(q4, 512)], qt[b * HPC][:, ts(q4, 512)]
                    )

                def _vx(q4):
                    nc.sync.dma_start(vx_sb[:, ts(q4, 4), :], vx[b][:, ts(q4, 4), :])

                if b == 0:
                    # at kernel start the g1 diag (key chunks 4-7) is the
                    # first thing to starve, so slices 0-1 of kt/qt jump the
                    # vx loads (PV consumers trail the QKs by two slots)
                    _kt_qt(0); _kt_qt(1); _vx(0); _vx(1)
                    _kt_qt(2); _vx(2); _kt_qt(3); _vx(3)
                else:
                    for q4 in range(4):
                        _kt_qt(q4); _vx(q4)
                pb_sb = kv_pool.tile([D, NCHUNK], F32, tag="pb")
                nc.gpsimd.dma_start(pb_sb[:], pb[b].rearrange("(j k) -> k j", k=128))
                kt_sbs[b], vx_sbs[b], pb_sbs[b] = kt_sb, vx_sb, pb_sb
                qt_sbs[b * HPC] = qt0_sb

            def _load_qt(pair):
                qt_sb = q_pool.tile([D, S], BF16, tag="qt")
                for q4 in range(4):
                    nc.sync.dma_start(
                        qt_sb[:, ts(q4, 512)], qt[pair][:, ts(q4, 512)]
                    )
                qt_sbs[pair] = qt_sb

            def emit_qk(bb):
                bb.st = st_pool.tile([D, 1536], F32, tag="st")
                qt_sb = qt_sbs[bb.pair]
                kt_sb = kt_sbs[bb.b]
                if bb.diag:
                    # modest priority boost: at equal readiness the scheduler
                    # must pick these over backlogged PVs or the diag exp
                    # (and the whole group handoff) slips by ~1us
                    with tc.high_priority(offset=40):
                        for u in range(4):
                            j = 4 * bb.g + u
                            off = DIAG_OFF[u]
                            w = 512 - 128 * u
                            nc.tensor.matmul(
                                bb.st[:, off : off + w],
                                lhsT=kt_sb[:, ts(j, 128)],
                                rhs=qt_sb[:, 512 * bb.g + 128 * u : 512 * (bb.g + 1)],
                                start=True,
                                stop=True,
                            )
                else:
                    for i, j in enumerate(bb.chunks):
                        nc.tensor.matmul(
                            bb.st[:, ts(i, 512)],
                            lhsT=kt_sb[:, ts(j, 128)],
                            rhs=qt_sb[:, ts(bb.g, 512)],
                            start=True,
                            stop=True,
                        )

            def emit_exp(bb):
                bb.pt = pt_pool.tile([D, 1536], BF16, tag="pt")
                pb_sb = pb_sbs[bb.b]
                if bb.diag:
                    if uniform_mask:
                        nc.scalar.activation(
                            bb.pt[:, :DIAG_COLS], bb.st[:, :DIAG_COLS], EXP,
                            scale=SCALE_ACT,
                        )
                    else:
                        for u in range(4):
                            j = 4 * bb.g + u
                            off = DIAG_OFF[u]
                            w = 512 - 128 * u
                            nc.scalar.activation(
                                bb.pt[:, off : off + w], bb.st[:, off : off + w],
                                EXP, bias=pb_sb[:, j : j + 1], scale=SCALE_ACT,
                            )
                    # zero the upper triangle of each diag chunk's first live
                    # 128-col slice (its t=u tile): P^T *= tri01
                    for u in range(4):
                        off = DIAG_OFF[u]
                        nc.vector.tensor_tensor(
                            bb.pt[:, off : off + 128],
                            bb.pt[:, off : off + 128],
                            tri_sb[:],
                            mybir.AluOpType.mult,
                        )
                elif bb.sch:
                    # Schraudolph exp on DVE: one add + f32->i16 convert;
                    # the i16 bits ARE the bf16 probabilities
                    with nc.allow_low_precision(
                        reason="approximate exp within validated tolerance"
                    ):
                        nc.vector.tensor_scalar(
                            bb.pt[:, :1536].bitcast(mybir.dt.int16),
                            bb.st[:, :1536],
                            float(B_SCH),
                            None,
                            mybir.AluOpType.add,
                        )
                else:
                    L = len(bb.chunks)
                    if uniform_mask:
                        nc.scalar.activation(
                            bb.pt[:, : 512 * L], bb.st[:, : 512 * L], EXP,
                            scale=SCALE_ACT,
                        )
                    else:
                        for i, j in enumerate(bb.chunks):
                            nc.scalar.activation(
                                bb.pt[:, ts(i, 512)], bb.st[:, ts(i, 512)], EXP,
                                bias=pb_sb[:, j : j + 1], scale=SCALE,
                            )

            def emit_pv(bb):
                if bb.first_of_group:
                    # each acc tile is exactly one PSUM bank holding 2 q-tiles
                    # of the group; [:, t, 128] is the softmax denominator
                    acc0 = acc_pool.tile([D, 2, 256], F32, tag="acc")
                    acc1 = acc_pool.tile([D, 2, 256], F32, tag="acc")
                    bb.grp = (acc0, acc1)
                    grp_accs[(bb.pair, bb.g)] = bb.grp
                acc0, acc1 = grp_accs[(bb.pair, bb.g)]
                vx_sb = vx_sbs[bb.b]
                g = bb.g
                for ci, j in enumerate(bb.chunks):
                    if bb.diag:
                        u = j - 4 * g
                        off = DIAG_OFF[u]
                        t_lo = u
                    else:
                        off = 512 * ci
                        t_lo = 0
                    first_j = bb.first_of_group and ci == 0
                    for t in range(t_lo, 4):
                        acc = acc0 if t < 2 else acc1
                        tsub = t % 2
                        if bb.diag:
                            pt_ap = bb.pt[
                                :, off + 128 * (t - t_lo) : off + 128 * (t - t_lo) + 128
                            ]
                        else:
                            pt_ap = bb.pt[:, off + 128 * t : off + 128 * t + 128]
                        # one start per bank (zeroes the whole 2KB zero
                        # region); one stop per bank on its last contributor:
                        # bank0's last is diag u1 (j=4g+1) t=1, bank1's last
                        # is diag u3 (j=4g+3) t=3.
                        st_flag = first_j and t in (0, 2)
                        if t < 2:
                            sp_flag = (j == 4 * g + 1) and t == 1
                        else:
                            sp_flag = (j == 4 * g + 3) and t == 3
                        nc.tensor.matmul(
                            acc[:, tsub, 0:129],
                            lhsT=pt_ap,
                            rhs=vx_sb[:, j, 0:129],
                            start=st_flag,
                            stop=sp_flag,
                        )

            def emit_norm(bb):
                acc0, acc1 = grp_accs.pop((bb.pair, bb.g))
                # copy PSUM accumulators out immediately (frees each bank for
                # the next group's start=True zeroing ~400ns after last PV);
                # recip+normalize then run off the critical path from SBUF
                oc = osb_pool.tile([D, 4, 129], F32, tag="oc")
                nc.vector.tensor_copy(oc[:, 0:2, :], acc0[:, :, 0:129])
                nc.vector.tensor_copy(oc[:, 2:4, :], acc1[:, :, 0:129])
                osb = osb_pool.tile([D, 4, D], BF16, tag="osb")
                rsum = small_pool.tile([D, 4, 1], F32, tag="rs")
                nc.vector.reciprocal(rsum[:], oc[:, :, 128:129])
                with nc.allow_low_precision(
                    reason="bf16 output rounding is within tolerance"
                ):
                    for t in range(4):
                        nc.vector.tensor_scalar(
                            osb[:, t, :],
                            oc[:, t, 0:128],
                            rsum[:, t, :],
                            None,
                            mybir.AluOpType.mult,
                        )
                nc.sync.dma_start(ot[bb.pair, bb.g], osb[:])

            grp_accs = {}
            _load_kv(0)

            # prologue actions keyed by emission slot: qt prefetches for
            # same-batch pairs ~6 batches early; batch-1 KV load during pair 2
            prologue = {}
            for pair in range(1, PAIRS):
                if pair == HPC:
                    continue  # loaded by _load_kv(1)
                prologue.setdefault(max(0, first_idx[pair] - 6), []).append(
                    lambda p=pair: _load_qt(p)
                )
            prologue.setdefault(first_idx[3], []).append(lambda: _load_kv(1))

            # 2-deep software pipeline: in the window where ACT runs exp(k),
            # the PE executes QK(k+1) first (data ready at window start, so
            # exp(k+1) is never gated by QK latency) and then PV(k-1) (whose
            # pt has been ready since exp(k-1) ended - no sem-latency stall).
            n = len(sched)
            for i in range(n + 2):
                for fn in prologue.get(i, []):
                    fn()
                if i < n:
                    emit_qk(sched[i])
                if 1 <= i <= n:
                    emit_exp(sched[i - 1])
                if 2 <= i:
                    bb = sched[i - 2]
                    emit_pv(bb)
                    if bb.last_of_group:
                        emit_norm(bb)

    nc.compile()
    return nc


_NC = {}


def _get_nc(uniform_mask: bool = True):
    if uniform_mask not in _NC:
        _NC[uniform_mask] = build_module(uniform_mask)
    return _NC[uniform_mask]


def shard_inputs(q, kv, key_padding_mask):
    """Full inputs -> list of 8 per-core input maps."""
    import ml_dtypes

    bf16 = ml_dtypes.bfloat16
    q = np.asarray(q, dtype=np.float32)
    kv = np.asarray(kv, dtype=np.float32)
    mask = np.asarray(key_padding_mask)

    pbias = np.where(mask, np.float32(0.0), np.float32(NEG)).astype(np.float32)

    # tri01[k, qq] = 1 where k <= qq (keys on partitions)
    kk = np.arange(128)[:, None]
    qq = np.arange(128)[None, :]
    tri01 = (kk <= qq).astype(bf16)

    in_maps = []
    for c in range(N_CORES):
        qc = q[:, :, HPC * c : HPC * (c + 1), :]  # [B, S, 4, D]
        qtc = (
            (np.ascontiguousarray(np.transpose(qc, (0, 2, 3, 1)))
             * np.float32(A_SCH))
            .reshape(PAIRS, D, S)
            .astype(bf16)
        )
        kc = kv[:, :, 0, c, :]  # [B, S, D]
        vc = kv[:, :, 1, c, :]  # [B, S, D]
        ktc = np.ascontiguousarray(np.transpose(kc, (0, 2, 1))).astype(bf16)
        # vx[b, k, j, 0:128] = v[b, 128j+k, :]; [..., 128] = 1; pad 0
        vxc = np.zeros((B, D, NCHUNK, VW), dtype=bf16)
        vxc[:, :, :, :128] = np.transpose(
            vc.reshape(B, NCHUNK, 128, D), (0, 2, 1, 3)
        )
        vxc[:, :, :, 128] = bf16(1.0)
        in_maps.append(
            {"qt": qtc, "kt": ktc, "vx": vxc, "tri": tri01, "pb": pbias}
        )
    return in_maps


def unshard_output(results):
    """Per-core 'ot' [PAIRS, NG, 128, 4, 128] -> full [B, S, H, D] fp32."""
    out = np.empty((B, S, H, D), dtype=np.float32)
    for c in range(N_CORES):
        otc = np.asarray(results[c]["ot"], dtype=np.float32)
        for pair in range(PAIRS):
            b, h = pair // HPC, HPC * c + pair % HPC
            # [NG, 128p, 4t, D] -> [NG, 4t, 128p, D] -> [S, D]
            out[b, :, h, :] = np.transpose(otc[pair], (0, 2, 1, 3)).reshape(S, D)
    return out


def kernel(q, kv, key_padding_mask):
    uniform = bool(np.asarray(key_padding_mask).all())
    nc = _get_nc(uniform)
    in_maps = shard_inputs(q, kv, key_padding_mask)
    res = run_bass_kernel_spmd(nc, in_maps, core_ids=list(range(N_CORES)))
    return unshard_output(res.results)


# revision 25
# speedup vs baseline: 1.0043x; 1.0005x over previous
"""Trainium2 Bass kernel: causal GQA attention.

Problem: B=2, Sq=Sk=2048, H=32, Hkv=8, D=128, fp32, causal + key-padding mask.

Sharding (8 cores): head-parallel. Core c takes q-heads [4c, 4c+4) for both
batches; those 4 heads share exactly one kv head (c) per batch, so each core
runs 8 independent (batch, head) pairs — K/V loaded once per batch, no comms.

All matmuls run in bf16 (1 PE cycle/row at ANY output width — unlike fp32r's
>=256 constraint — so diagonal chunks are sliced to exactly their live
columns). Scores are built TRANSPOSED (keys on partitions, queries on free) so
softmax-weight x V contracts the key axis directly with V in natural layout.

Work is organized as a flat, software-pipelined stream of "batches" (1-3 key
chunks sharing one score tile and one exp instruction) spanning all pairs and
query groups: batch i+1's QK matmuls are emitted BEFORE batch i's PV matmuls
so the PE always has independent work while the ACT engine (the bottleneck)
runs exp back-to-back.

Per (pair, 512-query group g): off-diagonal chunks in batches of 3; one
packed diagonal batch (4 chunks column-packed at offsets {0,512,896,1024} so
no matmul output crosses a PSUM bank and one exp covers all 1280 live cols):
    S^T = K_j @ Q_g^T            (PE bf16 -> PSUM fp32, diag sliced to live cols)
    P^T = exp(scale*S^T)         (ACT, one instr per batch, PSUM->SBUF bf16)
    diag triangles: P^T *= tri01 (DVE bf16, [128,128] per diag chunk)
    O/sums fused:  acc[t] += P^T[:, t-tile].T @ [V_j | 1]   (PE, 129-wide
                   moving operand: the appended ones column accumulates the
                   softmax denominator in the same matmul)
  acc layout: 2 PSUM banks per group, each holding 2 q-tiles [128, 129].
  One start=True per bank (zeroes the whole 2KB zero region) and one
  stop=True per bank; in-between matmuls accumulate with start=False.
  rsum = 1/acc[:, :, 128]        (DVE reciprocal)
  out  = acc[:, t, 0:128] * rsum[t]   (DVE tensor_scalar per-partition, ->bf16)
  DMA out [128q, 4t, 128d]; host reassembles (no transposes on device).

Two off-diag batches per pair (SCH_BATCHES) skip ACT entirely: with Q
pre-scaled by A_SCH on the host, exp becomes a single DVE tensor_scalar
(add B_SCH, convert f32->i16, bitcast bf16) - Schraudolph's approximation
in bf16, validated to ~9e-3 rel err even if applied everywhere.

PSUM: 2 x 3-bank rotating score tiles + 2 single-buffered accumulator banks
= all 8 banks. Cost-model engine busy: PE 118us, ACT 115us, DVE 86us of
146us total (baseline was 197us).
"""

import math
import sys

import numpy as np

for _p in ("/opt/trn_rl_repo",):
    if _p not in sys.path:
        sys.path.append(_p)

import concourse.bass as bass
import concourse.tile as tile
from concourse import bacc, mybir
from concourse.bass import ts
from concourse.bass_utils import run_bass_kernel_spmd

B = 2
S = 2048
H = 32
HKV = 8
D = 128
N_CORES = 8
HPC = H // N_CORES  # q heads per core = 4
PAIRS = B * HPC  # 8 (batch, head) pairs per core
NG = S // 512  # 4 q-groups of 512 per pair
NCHUNK = S // 128  # 16 key chunks of 128
VW = 132  # padded vx row: 128 v cols + 1 ones col + 3 pad
SCALE = 1.0 / math.sqrt(D)
NEG = -10000.0
# Schraudolph-in-bf16 exp for DVE-offloaded batches: host pre-scales Q by
# A_SCH so st = A_SCH * (q.k); then bf16(exp(scale*q.k)) ~= bitcast_i16(
# st + B_SCH) and the exact-exp ACT batches just use scale SCALE_ACT.
A_SCH = SCALE * 128.0 * math.log2(math.e)
B_SCH = 127.0 * 128.0 - 486411.0 / 65536.0 + 0.5  # +0.5: f32->i16 truncates
SCALE_ACT = SCALE / A_SCH  # = ln2/128
# off-diag 3-chunk batches per (g, index-within-g) offloaded to DVE exp
SCH_BATCHES = {(3, 1)}

# column offset of diagonal chunk u inside the packed [128, 1280] score tile;
# widths are 512-128u, arranged so no matmul output crosses a 2KB PSUM bank:
# bank0: u0 (512 cols); bank1: u1 (384) + u3 (128); bank2: u2 (256)
DIAG_OFF = (0, 512, 1024, 896)
DIAG_COLS = 1280

F32 = mybir.dt.float32
BF16 = mybir.dt.bfloat16
EXP = mybir.ActivationFunctionType.Exp


class _Batch:
    __slots__ = (
        "pair", "b", "g", "chunks", "diag", "first_of_group", "last_of_group",
        "first_of_pair", "st", "pt", "grp", "sch",
    )

    def __init__(self, pair, b, g, chunks, diag, first_of_group, last_of_group):
        self.sch = False
        self.pair = pair
        self.b = b
        self.g = g
        self.chunks = chunks  # list of key-chunk indices j
        self.diag = diag
        self.first_of_group = first_of_group
        self.last_of_group = last_of_group
        self.st = None
        self.pt = None
        self.grp = None  # (acc0, acc1) for this (pair, g)


def build_module(uniform_mask: bool = True):
    nc = bacc.Bacc("TRN2", target_bir_lowering=False, debug=False, num_devices=1)

    qt = nc.dram_tensor("qt", [PAIRS, D, S], BF16, kind="ExternalInput").ap()
    kt = nc.dram_tensor("kt", [B, D, S], BF16, kind="ExternalInput").ap()
    vx = nc.dram_tensor("vx", [B, D, NCHUNK, VW], BF16, kind="ExternalInput").ap()
    tri = nc.dram_tensor("tri", [D, D], BF16, kind="ExternalInput").ap()
    pb = nc.dram_tensor("pb", [B, S], F32, kind="ExternalInput").ap()
    ot = nc.dram_tensor("ot", [PAIRS, NG, D, 4, D], BF16, kind="ExternalOutput").ap()

    # flat batch schedule for the whole core
    sched = []
    for b in range(B):
        for h in range(HPC):
            pair = b * HPC + h
            # the final pair runs its groups largest-first: the pipeline
            # drain after the last exp then only carries g0's small diag
            # PV + normalize instead of g3's full backlog
            g_order = range(NG - 1, -1, -1) if pair == PAIRS - 1 else range(NG)
            for g in g_order:
                nfull = 4 * g
                # balanced batch sizes (max 3 chunks), smallest first: a
                # short exp window right before the diag handoff compounds PE
                # overflow at the least elastic point; early, its overflow is
                # absorbed by later windows' slack
                groups = []
                if nfull:
                    nb_ = -(-nfull // 3)
                    base, rem = divmod(nfull, nb_)
                    sizes = [base] * (nb_ - rem) + [base + 1] * rem
                    s0 = 0
                    for sz in sizes:
                        groups.append(list(range(s0, s0 + sz)))
                        s0 += sz
                for bi, ch in enumerate(groups):
                    nb = _Batch(pair, b, g, ch, False, bi == 0, False)
                    nb.sch = (
                        uniform_mask
                        and (g, bi) in SCH_BATCHES
                        and len(ch) == 3
                        # the final pair's last sch batch would land in the
                        # drain tail where ACT idle is free anyway
                        and not (pair == PAIRS - 1 and (g, bi) == (3, 3))
                    )
                    sched.append(nb)
                sched.append(
                    _Batch(
                        pair, b, g, [4 * g + u for u in range(4)], True,
                        len(groups) == 0, True,
                    )
                )
    first_idx = {}  # pair -> index of its first batch
    for i, bb in enumerate(sched):
        if bb.pair not in first_idx:
            first_idx[bb.pair] = i

    with tile.TileContext(nc) as tc:
        with (
            tc.tile_pool(name="consts", bufs=1) as consts,
            tc.tile_pool(name="kv", bufs=2) as kv_pool,
            tc.tile_pool(name="q", bufs=3) as q_pool,
            tc.tile_pool(name="pt", bufs=6) as pt_pool,
            tc.tile_pool(name="osb", bufs=4) as osb_pool,
            tc.tile_pool(name="small", bufs=6) as small_pool,
            tc.tile_pool(name="st_ps", bufs=2, space="PSUM") as st_pool,
            tc.tile_pool(name="acc_ps", bufs=2, space="PSUM") as acc_pool,
        ):
            tri_sb = consts.tile([D, D], BF16)
            nc.sync.dma_start(tri_sb[:], tri[:])
            # warm the ACT exp table during the initial DMAs
            warm_i = consts.tile([1, 2], F32)
            warm_o = consts.tile([1, 2], F32)
            nc.vector.memset(warm_i[:], 0.0)
            nc.scalar.activation(warm_o[:], warm_i[:], EXP, scale=1.0)

            kt_sbs, vx_sbs, pb_sbs, qt_sbs = {}, {}, {}, {}

            def _load_kv(b):
                # split loads so the first group's compute starts after the
                # first slices. At kernel start (b=0) the DMA generation
                # latency is on the critical path, so spread the three
                # streams across all three DGE queues (sync/scalar/gpsimd);
                # mid-stream (b=1) keep everything off the ACT sequencer.
                kt_sb = kv_pool.tile([D, S], BF16, tag="kt")
                vx_sb = kv_pool.tile([D, NCHUNK, VW], BF16, tag="vx")
                qt0_sb = q_pool.tile([D, S], BF16, tag="qt")
                def _kt_qt(q4):
                    nc.sync.dma_start(kt_sb[:, ts(q4, 512)], kt[b][:, ts(q4, 512)])
                    nc.sync.dma_start(
                        qt0_sb[:, ts(q4, 512)], qt[b * HPC][:, ts(q4, 512)]
                    )

                def _vx(q4):
                    nc.sync.dma_start(vx_sb[:, ts(q4, 4), :], vx[b][:, ts(q4, 4), :])

                if b == 0:
                    # at kernel start the g1 diag (key chunks 4-7) is the
                    # first thing to starve, so slices 0-1 of kt/qt jump the
                    # vx loads (PV consumers trail the QKs by two slots)
                    _kt_qt(0); _kt_qt(1); _vx(0); _vx(1)
                    _kt_qt(2); _vx(2); _kt_qt(3); _vx(3)
                else:
                    for q4 in range(4):
                        _kt_qt(q4); _vx(q4)
                pb_sb = kv_pool.tile([D, NCHUNK], F32, tag="pb")
                nc.gpsimd.dma_start(pb_sb[:], pb[b].rearrange("(j k) -> k j", k=128))
                kt_sbs[b], vx_sbs[b], pb_sbs[b] = kt_sb, vx_sb, pb_sb
                qt_sbs[b * HPC] = qt0_sb

            def _load_qt(pair):
                qt_sb = q_pool.tile([D, S], BF16, tag="qt")
                for q4 in range(4):
                    nc.sync.dma_start(
                        qt_sb[:, ts(q4, 512)], qt[pair][:, ts(q4, 512)]
                    )
                qt_sbs[pair] = qt_sb

            def emit_qk(bb):
                bb.st = st_pool.tile([D, 1536], F32, tag="st")
                qt_sb = qt_sbs[bb.pair]
                kt_sb = kt_sbs[bb.b]
                if bb.diag:
                    # modest priority boost: at equal readiness the scheduler
                    # must pick these over backlogged PVs or the diag exp
                    # (and the whole group handoff) slips by ~1us
                    with tc.high_priority(offset=40):
                        for u in range(4):
                            j = 4 * bb.g + u
                            off = DIAG_OFF[u]
                            w = 512 - 128 * u
                            nc.tensor.matmul(
                                bb.st[:, off : off + w],
                                lhsT=kt_sb[:, ts(j, 128)],
                                rhs=qt_sb[:, 512 * bb.g + 128 * u : 512 * (bb.g + 1)],
                                start=True,
                                stop=True,
                            )
                else:
                    for i, j in enumerate(bb.chunks):
                        nc.tensor.matmul(
                            bb.st[:, ts(i, 512)],
                            lhsT=kt_sb[:, ts(j, 128)],
                            rhs=qt_sb[:, ts(bb.g, 512)],
                            start=True,
                            stop=True,
                        )

            def emit_exp(bb):
                bb.pt = pt_pool.tile([D, 1536], BF16, tag="pt")
                pb_sb = pb_sbs[bb.b]
                if bb.diag:
                    if uniform_mask:
                        nc.scalar.activation(
                            bb.pt[:, :DIAG_COLS], bb.st[:, :DIAG_COLS], EXP,
                            scale=SCALE_ACT,
                        )
                    else:
                        for u in range(4):
                            j = 4 * bb.g + u
                            off = DIAG_OFF[u]
                            w = 512 - 128 * u
                            nc.scalar.activation(
                                bb.pt[:, off : off + w], bb.st[:, off : off + w],
                                EXP, bias=pb_sb[:, j : j + 1], scale=SCALE_ACT,
                            )
                    # zero the upper triangle of each diag chunk's first live
                    # 128-col slice (its t=u tile): P^T *= tri01
                    for u in range(4):
                        off = DIAG_OFF[u]
                        nc.vector.tensor_tensor(
                            bb.pt[:, off : off + 128],
                            bb.pt[:, off : off + 128],
                            tri_sb[:],
                            mybir.AluOpType.mult,
                        )
                elif bb.sch:
                    # Schraudolph exp on DVE: one add + f32->i16 convert;
                    # the i16 bits ARE the bf16 probabilities
                    with nc.allow_low_precision(
                        reason="approximate exp within validated tolerance"
                    ):
                        nc.vector.tensor_scalar(
                            bb.pt[:, :1536].bitcast(mybir.dt.int16),
                            bb.st[:, :1536],
                            float(B_SCH),
                            None,
                            mybir.AluOpType.add,
                        )
                else:
                    L = len(bb.chunks)
                    if uniform_mask:
                        nc.scalar.activation(
                            bb.pt[:, : 512 * L], bb.st[:, : 512 * L], EXP,
                            scale=SCALE_ACT,
                        )
                    else:
                        for i, j in enumerate(bb.chunks):
                            nc.scalar.activation(
                                bb.pt[:, ts(i, 512)], bb.st[:, ts(i, 512)], EXP,
                                bias=pb_sb[:, j : j + 1], scale=SCALE,
                            )

            def emit_pv(bb):
                if bb.first_of_group:
                    # each acc tile is exactly one PSUM bank holding 2 q-tiles
                    # of the group; [:, t, 128] is the softmax denominator
                    acc0 = acc_pool.tile([D, 2, 256], F32, tag="acc")
                    acc1 = acc_pool.tile([D, 2, 256], F32, tag="acc")
                    bb.grp = (acc0, acc1)
                    grp_accs[(bb.pair, bb.g)] = bb.grp
                acc0, acc1 = grp_accs[(bb.pair, bb.g)]
                vx_sb = vx_sbs[bb.b]
                g = bb.g
                for ci, j in enumerate(bb.chunks):
                    if bb.diag:
                        u = j - 4 * g
                        off = DIAG_OFF[u]
                        t_lo = u
                    else:
                        off = 512 * ci
                        t_lo = 0
                    first_j = bb.first_of_group and ci == 0
                    for t in range(t_lo, 4):
                        acc = acc0 if t < 2 else acc1
                        tsub = t % 2
                        if bb.diag:
                            pt_ap = bb.pt[
                                :, off + 128 * (t - t_lo) : off + 128 * (t - t_lo) + 128
                            ]
                        else:
                            pt_ap = bb.pt[:, off + 128 * t : off + 128 * t + 128]
                        # one start per bank (zeroes the whole 2KB zero
                        # region); one stop per bank on its last contributor:
                        # bank0's last is diag u1 (j=4g+1) t=1, bank1's last
                        # is diag u3 (j=4g+3) t=3.
                        st_flag = first_j and t in (0, 2)
                        if t < 2:
                            sp_flag = (j == 4 * g + 1) and t == 1
                        else:
                            sp_flag = (j == 4 * g + 3) and t == 3
                        nc.tensor.matmul(
                            acc[:, tsub, 0:129],
                            lhsT=pt_ap,
                            rhs=vx_sb[:, j, 0:129],
                            start=st_flag,
                            stop=sp_flag,
                        )

            def emit_norm(bb):
                acc0, acc1 = grp_accs.pop((bb.pair, bb.g))
                # copy PSUM accumulators out immediately (frees each bank for
                # the next group's start=True zeroing ~400ns after last PV);
                # recip+normalize then run off the critical path from SBUF
                oc = osb_pool.tile([D, 4, 129], F32, tag="oc")
                nc.vector.tensor_copy(oc[:, 0:2, :], acc0[:, :, 0:129])
                nc.vector.tensor_copy(oc[:, 2:4, :], acc1[:, :, 0:129])
                osb = osb_pool.tile([D, 4, D], BF16, tag="osb")
                rsum = small_pool.tile([D, 4, 1], F32, tag="rs")
                nc.vector.reciprocal(rsum[:], oc[:, :, 128:129])
                with nc.allow_low_precision(
                    reason="bf16 output rounding is within tolerance"
                ):
                    for t in range(4):
                        nc.vector.tensor_scalar(
                            osb[:, t, :],
                            oc[:, t, 0:128],
                            rsum[:, t, :],
                            None,
                            mybir.AluOpType.mult,
                        )
                nc.sync.dma_start(ot[bb.pair, bb.g], osb[:])

            grp_accs = {}
            _load_kv(0)

            # prologue actions keyed by emission slot: qt prefetches for
            # same-batch pairs ~6 batches early; batch-1 KV load during pair 2
            prologue = {}
            for pair in range(1, PAIRS):
                if pair == HPC:
                    continue  # loaded by _load_kv(1)
                prologue.setdefault(max(0, first_idx[pair] - 6), []).append(
                    lambda p=pair: _load_qt(p)
                )
            prologue.setdefault(first_idx[3], []).append(lambda: _load_kv(1))

            # 2-deep software pipeline: in the window where ACT runs exp(k),
            # the PE executes QK(k+1) first (data ready at window start, so
            # exp(k+1) is never gated by QK latency) and then PV(k-1) (whose
            # pt has been ready since exp(k-1) ended - no sem-latency stall).
            n = len(sched)
            for i in range(n + 2):
                for fn in prologue.get(i, []):
                    fn()
                if i < n:
                    emit_qk(sched[i])
                if 1 <= i <= n:
                    emit_exp(sched[i - 1])
                if 2 <= i:
                    bb = sched[i - 2]
                    emit_pv(bb)
                    if bb.last_of_group:
                        emit_norm(bb)

    nc.compile()
    return nc


_NC = {}


def _get_nc(uniform_mask: bool = True):
    if uniform_mask not in _NC:
        _NC[uniform_mask] = build_module(uniform_mask)
    return _NC[uniform_mask]


def shard_inputs(q, kv, key_padding_mask):
    """Full inputs -> list of 8 per-core input maps."""
    import ml_dtypes

    bf16 = ml_dtypes.bfloat16
    q = np.asarray(q, dtype=np.float32)
    kv = np.asarray(kv, dtype=np.float32)
    mask = np.asarray(key_padding_mask)

    pbias = np.where(mask, np.float32(0.0), np.float32(NEG)).astype(np.float32)

    # tri01[k, qq] = 1 where k <= qq (keys on partitions)
    kk = np.arange(128)[:, None]
    qq = np.arange(128)[None, :]
    tri01 = (kk <= qq).astype(bf16)

    in_maps = []
    for c in range(N_CORES):
        qc = q[:, :, HPC * c : HPC * (c + 1), :]  # [B, S, 4, D]
        qtc = (
            (np.ascontiguousarray(np.transpose(qc, (0, 2, 3, 1)))
             * np.float32(A_SCH))
            .reshape(PAIRS, D, S)
            .astype(bf16)
        )
        kc = kv[:, :, 0, c, :]  # [B, S, D]
        vc = kv[:, :, 1, c, :]  # [B, S, D]
        ktc = np.ascontiguousarray(np.transpose(kc, (0, 2, 1))).astype(bf16)
        # vx[b, k, j, 0:128] = v[b, 128j+k, :]; [..., 128] = 1; pad 0
        vxc = np.zeros((B, D, NCHUNK, VW), dtype=bf16)
        vxc[:, :, :, :128] = np.transpose(
            vc.reshape(B, NCHUNK, 128, D), (0, 2, 1, 3)
        )
        vxc[:, :, :, 128] = bf16(1.0)
        in_maps.append(
            {"qt": qtc, "kt": ktc, "vx": vxc, "tri": tri01, "pb": pbias}
        )
    return in_maps


def unshard_output(results):
    """Per-core 'ot' [PAIRS, NG, 128, 4, 128] -> full [B, S, H, D] fp32."""
    out = np.empty((B, S, H, D), dtype=np.float32)
    for c in range(N_CORES):
        otc = np.asarray(results[c]["ot"], dtype=np.float32)
        for pair in range(PAIRS):
            b, h = pair // HPC, HPC * c + pair % HPC
            # [NG, 128p, 4t, D] -> [NG, 4t, 128p, D] -> [S, D]
            out[b, :, h, :] = np.transpose(otc[pair], (0, 2, 1, 3)).reshape(S, D)
    return out


def kernel(q, kv, key_padding_mask):
    uniform = bool(np.asarray(key_padding_mask).all())
    nc = _get_nc(uniform)
    in_maps = shard_inputs(q, kv, key_padding_mask)
    res = run_bass_kernel_spmd(nc, in_maps, core_ids=list(range(N_CORES)))
    return unshard_output(res.results)


# revision 27
# speedup vs baseline: 1.0184x; 1.0141x over previous
"""Trainium2 Bass kernel: causal GQA attention.

Problem: B=2, Sq=Sk=2048, H=32, Hkv=8, D=128, fp32, causal + key-padding mask.

Sharding (8 cores): head-parallel. Core c takes q-heads [4c, 4c+4) for both
batches; those 4 heads share exactly one kv head (c) per batch, so each core
runs 8 independent (batch, head) pairs — K/V loaded once per batch, no comms.

All matmuls run in bf16 (1 PE cycle/row at ANY output width — unlike fp32r's
>=256 constraint — so diagonal chunks are sliced to exactly their live
columns). Scores are built TRANSPOSED (keys on partitions, queries on free) so
softmax-weight x V contracts the key axis directly with V in natural layout.

Work is organized as a flat, software-pipelined stream of "batches" (1-3 key
chunks sharing one score tile and one exp instruction) spanning all pairs and
query groups: batch i+1's QK matmuls are emitted BEFORE batch i's PV matmuls
so the PE always has independent work while the ACT engine (the bottleneck)
runs exp back-to-back.

Per (pair, 512-query group g): off-diagonal chunks in batches of 3; one
packed diagonal batch (4 chunks column-packed at offsets {0,512,896,1024} so
no matmul output crosses a PSUM bank and one exp covers all 1280 live cols):
    S^T = K_j @ Q_g^T            (PE bf16 -> PSUM fp32, diag sliced to live cols)
    P^T = exp(scale*S^T)         (ACT, one instr per batch, PSUM->SBUF bf16)
    diag triangles: P^T *= tri01 (DVE bf16, [128,128] per diag chunk)
    O/sums fused:  acc[t] += P^T[:, t-tile].T @ [V_j | 1]   (PE, 129-wide
                   moving operand: the appended ones column accumulates the
                   softmax denominator in the same matmul)
  acc layout: 2 PSUM banks per group, each holding 2 q-tiles [128, 129].
  One start=True per bank (zeroes the whole 2KB zero region) and one
  stop=True per bank; in-between matmuls accumulate with start=False.
  rsum = 1/acc[:, :, 128]        (DVE reciprocal)
  out  = acc[:, t, 0:128] * rsum[t]   (DVE tensor_scalar per-partition, ->bf16)
  DMA out [128q, 4t, 128d]; host reassembles (no transposes on device).

One off-diag batch per pair (SCH_BATCHES) skips ACT entirely: with Q
pre-scaled by A_SCH on the host, exp becomes a single DVE tensor_scalar
(add B_SCH, convert f32->i16, bitcast bf16) - Schraudolph's approximation
in bf16, validated to ~9e-3 rel err even if applied everywhere. The final
pair runs its groups largest-first so the pipeline drain ends on g0's small
diag; at startup kt/qt slices 0-1 jump ahead of the vx loads.

PSUM: 2 x 3-bank rotating score tiles + 2 single-buffered accumulator banks
= all 8 banks. Cost-model engine busy: PE ~118us, ACT ~116us, DVE ~80us of
145.6us total (baseline was 197us).
"""

import math
import sys

import numpy as np

for _p in ("/opt/trn_rl_repo",):
    if _p not in sys.path:
        sys.path.append(_p)

import concourse.bass as bass
import concourse.tile as tile
from concourse import bacc, mybir
from concourse.bass import ts
from concourse.bass_utils import run_bass_kernel_spmd

B = 2
S = 2048
H = 32
HKV = 8
D = 128
N_CORES = 8
HPC = H // N_CORES  # q heads per core = 4
PAIRS = B * HPC  # 8 (batch, head) pairs per core
NG = S // 512  # 4 q-groups of 512 per pair
NCHUNK = S // 128  # 16 key chunks of 128
VW = 132  # padded vx row: 128 v cols + 1 ones col + 3 pad
SCALE = 1.0 / math.sqrt(D)
NEG = -10000.0
# Schraudolph-in-bf16 exp for DVE-offloaded batches: host pre-scales Q by
# A_SCH so st = A_SCH * (q.k); then bf16(exp(scale*q.k)) ~= bitcast_i16(
# st + B_SCH) and the exact-exp ACT batches just use scale SCALE_ACT.
A_SCH = SCALE * 128.0 * math.log2(math.e)
B_SCH = 127.0 * 128.0 - 486411.0 / 65536.0 + 0.5  # +0.5: f32->i16 truncates
SCALE_ACT = SCALE / A_SCH  # = ln2/128
# off-diag 3-chunk batches per (g, index-within-g) offloaded to DVE exp
SCH_BATCHES = {(3, 1)}

# column offset of diagonal chunk u inside the packed [128, 1280] score tile;
# widths are 512-128u, arranged so no matmul output crosses a 2KB PSUM bank:
# bank0: u0 (512 cols); bank1: u1 (384) + u3 (128); bank2: u2 (256)
DIAG_OFF = (0, 512, 1024, 896)
DIAG_COLS = 1280

F32 = mybir.dt.float32
BF16 = mybir.dt.bfloat16
EXP = mybir.ActivationFunctionType.Exp


class _Batch:
    __slots__ = (
        "pair", "b", "g", "chunks", "diag", "first_of_group", "last_of_group",
        "first_of_pair", "st", "pt", "grp", "sch",
    )

    def __init__(self, pair, b, g, chunks, diag, first_of_group, last_of_group):
        self.sch = False
        self.pair = pair
        self.b = b
        self.g = g
        self.chunks = chunks  # list of key-chunk indices j
        self.diag = diag
        self.first_of_group = first_of_group
        self.last_of_group = last_of_group
        self.st = None
        self.pt = None
        self.grp = None  # (acc0, acc1) for this (pair, g)


def build_module(uniform_mask: bool = True):
    nc = bacc.Bacc("TRN2", target_bir_lowering=False, debug=False, num_devices=1)

    qt = nc.dram_tensor("qt", [PAIRS, D, S], BF16, kind="ExternalInput").ap()
    kt = nc.dram_tensor("kt", [B, D, S], BF16, kind="ExternalInput").ap()
    vx = nc.dram_tensor("vx", [B, D, NCHUNK, VW], BF16, kind="ExternalInput").ap()
    tri = nc.dram_tensor("tri", [D, D], BF16, kind="ExternalInput").ap()
    pb = nc.dram_tensor("pb", [B, S], F32, kind="ExternalInput").ap()
    ot = nc.dram_tensor("ot", [PAIRS, NG, D, 4, D], BF16, kind="ExternalOutput").ap()

    # flat batch schedule for the whole core
    sched = []
    for b in range(B):
        for h in range(HPC):
            pair = b * HPC + h
            # the final pair runs its groups largest-first: the pipeline
            # drain after the last exp then only carries g0's small diag
            # PV + normalize instead of g3's full backlog
            g_order = range(NG - 1, -1, -1) if pair == PAIRS - 1 else range(NG)
            for g in g_order:
                nfull = 4 * g
                # balanced batch sizes (max 3 chunks), smallest first: a
                # short exp window right before the diag handoff compounds PE
                # overflow at the least elastic point; early, its overflow is
                # absorbed by later windows' slack
                groups = []
                if nfull:
                    nb_ = -(-nfull // 3)
                    base, rem = divmod(nfull, nb_)
                    sizes = [base] * (nb_ - rem) + [base + 1] * rem
                    s0 = 0
                    for sz in sizes:
                        groups.append(list(range(s0, s0 + sz)))
                        s0 += sz
                for bi, ch in enumerate(groups):
                    nb = _Batch(pair, b, g, ch, False, bi == 0, False)
                    nb.sch = (
                        uniform_mask
                        and (g, bi) in SCH_BATCHES
                        and len(ch) == 3
                        # the final pair's last sch batch would land in the
                        # drain tail where ACT idle is free anyway
                        and not (pair == PAIRS - 1 and (g, bi) == (3, 3))
                    )
                    sched.append(nb)
                sched.append(
                    _Batch(
                        pair, b, g, [4 * g + u for u in range(4)], True,
                        len(groups) == 0, True,
                    )
                )
    first_idx = {}  # pair -> index of its first batch
    for i, bb in enumerate(sched):
        if bb.pair not in first_idx:
            first_idx[bb.pair] = i

    with tile.TileContext(nc) as tc:
        with (
            tc.tile_pool(name="consts", bufs=1) as consts,
            tc.tile_pool(name="kv", bufs=2) as kv_pool,
            tc.tile_pool(name="q", bufs=4) as q_pool,
            tc.tile_pool(name="pt", bufs=8) as pt_pool,
            tc.tile_pool(name="osb", bufs=4) as osb_pool,
            tc.tile_pool(name="small", bufs=6) as small_pool,
            tc.tile_pool(name="st_ps", bufs=2, space="PSUM") as st_pool,
            tc.tile_pool(name="acc_ps", bufs=2, space="PSUM") as acc_pool,
        ):
            tri_sb = consts.tile([D, D], BF16)
            nc.gpsimd.dma_start(tri_sb[:], tri[:])
            # warm the ACT exp table during the initial DMAs
            warm_i = consts.tile([1, 2], F32)
            warm_o = consts.tile([1, 2], F32)
            nc.vector.memset(warm_i[:], 0.0)
            nc.scalar.activation(warm_o[:], warm_i[:], EXP, scale=1.0)

            kt_sbs, vx_sbs, pb_sbs, qt_sbs = {}, {}, {}, {}

            def _load_kv(b):
                # split loads so the first group's compute starts after the
                # first slices. At kernel start (b=0) the DMA generation
                # latency is on the critical path, so spread the three
                # streams across all three DGE queues (sync/scalar/gpsimd);
                # mid-stream (b=1) keep everything off the ACT sequencer.
                kt_sb = kv_pool.tile([D, S], BF16, tag="kt")
                vx_sb = kv_pool.tile([D, NCHUNK, VW], BF16, tag="vx")
                qt0_sb = q_pool.tile([D, S], BF16, tag="qt")
                def _kt_qt(q4):
                    nc.sync.dma_start(kt_sb[:, ts(q4, 512)], kt[b][:, ts(q4, 512)])
                    nc.sync.dma_start(
                        qt0_sb[:, ts(q4, 512)], qt[b * HPC][:, ts(q4, 512)]
                    )

                def _vx(q4):
                    nc.sync.dma_start(vx_sb[:, ts(q4, 4), :], vx[b][:, ts(q4, 4), :])

                if b == 0:
                    # at kernel start the g1 diag (key chunks 4-7) is the
                    # first thing to starve, so slices 0-1 of kt/qt jump the
                    # vx loads (PV consumers trail the QKs by two slots)
                    _kt_qt(0); _kt_qt(1); _vx(0); _vx(1)
                    _kt_qt(2); _vx(2); _kt_qt(3); _vx(3)
                else:
                    for q4 in range(4):
                        _kt_qt(q4); _vx(q4)
                pb_sb = kv_pool.tile([D, NCHUNK], F32, tag="pb")
                nc.gpsimd.dma_start(pb_sb[:], pb[b].rearrange("(j k) -> k j", k=128))
                kt_sbs[b], vx_sbs[b], pb_sbs[b] = kt_sb, vx_sb, pb_sb
                qt_sbs[b * HPC] = qt0_sb

            def _load_qt(pair):
                qt_sb = q_pool.tile([D, S], BF16, tag="qt")
                for q4 in range(4):
                    nc.sync.dma_start(
                        qt_sb[:, ts(q4, 512)], qt[pair][:, ts(q4, 512)]
                    )
                qt_sbs[pair] = qt_sb

            def emit_qk(bb):
                bb.st = st_pool.tile([D, 1536], F32, tag="st")
                qt_sb = qt_sbs[bb.pair]
                kt_sb = kt_sbs[bb.b]
                if bb.diag:
                    # modest priority boost: at equal readiness the scheduler
                    # must pick these over backlogged PVs or the diag exp
                    # (and the whole group handoff) slips by ~1us
                    with tc.high_priority(offset=40):
                        for u in range(4):
                            j = 4 * bb.g + u
                            off = DIAG_OFF[u]
                            w = 512 - 128 * u
                            nc.tensor.matmul(
                                bb.st[:, off : off + w],
                                lhsT=kt_sb[:, ts(j, 128)],
                                rhs=qt_sb[:, 512 * bb.g + 128 * u : 512 * (bb.g + 1)],
                                start=True,
                                stop=True,
                            )
                else:
                    for i, j in enumerate(bb.chunks):
                        nc.tensor.matmul(
                            bb.st[:, ts(i, 512)],
                            lhsT=kt_sb[:, ts(j, 128)],
                            rhs=qt_sb[:, ts(bb.g, 512)],
                            start=True,
                            stop=True,
                        )

            def emit_exp(bb):
                bb.pt = pt_pool.tile([D, 1536], BF16, tag="pt")
                pb_sb = pb_sbs[bb.b]
                if bb.diag:
                    if uniform_mask:
                        nc.scalar.activation(
                            bb.pt[:, :DIAG_COLS], bb.st[:, :DIAG_COLS], EXP,
                            scale=SCALE_ACT,
                        )
                    else:
                        for u in range(4):
                            j = 4 * bb.g + u
                            off = DIAG_OFF[u]
                            w = 512 - 128 * u
                            nc.scalar.activation(
                                bb.pt[:, off : off + w], bb.st[:, off : off + w],
                                EXP, bias=pb_sb[:, j : j + 1], scale=SCALE_ACT,
                            )
                    # zero the upper triangle of each diag chunk's first live
                    # 128-col slice (its t=u tile): P^T *= tri01
                    for u in range(4):
                        off = DIAG_OFF[u]
                        nc.vector.tensor_tensor(
                            bb.pt[:, off : off + 128],
                            bb.pt[:, off : off + 128],
                            tri_sb[:],
                            mybir.AluOpType.mult,
                        )
                elif bb.sch:
                    # Schraudolph exp on DVE: one add + f32->i16 convert;
                    # the i16 bits ARE the bf16 probabilities
                    with nc.allow_low_precision(
                        reason="approximate exp within validated tolerance"
                    ):
                        nc.vector.tensor_scalar(
                            bb.pt[:, :1536].bitcast(mybir.dt.int16),
                            bb.st[:, :1536],
                            float(B_SCH),
                            None,
                            mybir.AluOpType.add,
                        )
                else:
                    L = len(bb.chunks)
                    if uniform_mask:
                        nc.scalar.activation(
                            bb.pt[:, : 512 * L], bb.st[:, : 512 * L], EXP,
                            scale=SCALE_ACT,
                        )
                    else:
                        for i, j in enumerate(bb.chunks):
                            nc.scalar.activation(
                                bb.pt[:, ts(i, 512)], bb.st[:, ts(i, 512)], EXP,
                                bias=pb_sb[:, j : j + 1], scale=SCALE,
                            )

            def emit_pv(bb):
                if bb.first_of_group:
                    # each acc tile is exactly one PSUM bank holding 2 q-tiles
                    # of the group; [:, t, 128] is the softmax denominator
                    acc0 = acc_pool.tile([D, 2, 256], F32, tag="acc")
                    acc1 = acc_pool.tile([D, 2, 256], F32, tag="acc")
                    bb.grp = (acc0, acc1)
                    grp_accs[(bb.pair, bb.g)] = bb.grp
                acc0, acc1 = grp_accs[(bb.pair, bb.g)]
                vx_sb = vx_sbs[bb.b]
                g = bb.g
                for ci, j in enumerate(bb.chunks):
                    if bb.diag:
                        u = j - 4 * g
                        off = DIAG_OFF[u]
                        t_lo = u
                    else:
                        off = 512 * ci
                        t_lo = 0
                    first_j = bb.first_of_group and ci == 0
                    for t in range(t_lo, 4):
                        acc = acc0 if t < 2 else acc1
                        tsub = t % 2
                        if bb.diag:
                            pt_ap = bb.pt[
                                :, off + 128 * (t - t_lo) : off + 128 * (t - t_lo) + 128
                            ]
                        else:
                            pt_ap = bb.pt[:, off + 128 * t : off + 128 * t + 128]
                        # one start per bank (zeroes the whole 2KB zero
                        # region); one stop per bank on its last contributor:
                        # bank0's last is diag u1 (j=4g+1) t=1, bank1's last
                        # is diag u3 (j=4g+3) t=3.
                        st_flag = first_j and t in (0, 2)
                        if t < 2:
                            sp_flag = (j == 4 * g + 1) and t == 1
                        else:
                            sp_flag = (j == 4 * g + 3) and t == 3
                        nc.tensor.matmul(
                            acc[:, tsub, 0:129],
                            lhsT=pt_ap,
                            rhs=vx_sb[:, j, 0:129],
                            start=st_flag,
                            stop=sp_flag,
                        )

            def emit_norm(bb):
                acc0, acc1 = grp_accs.pop((bb.pair, bb.g))
                # copy PSUM accumulators out immediately (frees each bank for
                # the next group's start=True zeroing ~400ns after last PV);
                # recip+normalize then run off the critical path from SBUF
                oc = osb_pool.tile([D, 4, 129], F32, tag="oc")
                nc.vector.tensor_copy(oc[:, 0:2, :], acc0[:, :, 0:129])
                nc.vector.tensor_copy(oc[:, 2:4, :], acc1[:, :, 0:129])
                osb = osb_pool.tile([D, 4, D], BF16, tag="osb")
                rsum = small_pool.tile([D, 4, 1], F32, tag="rs")
                nc.vector.reciprocal(rsum[:], oc[:, :, 128:129])
                with nc.allow_low_precision(
                    reason="bf16 output rounding is within tolerance"
                ):
                    for t in range(4):
                        nc.vector.tensor_scalar(
                            osb[:, t, :],
                            oc[:, t, 0:128],
                            rsum[:, t, :],
                            None,
                            mybir.AluOpType.mult,
                        )
                nc.sync.dma_start(ot[bb.pair, bb.g], osb[:])

            grp_accs = {}
            _load_kv(0)

            # prologue actions keyed by emission slot: qt prefetches for
            # same-batch pairs ~6 batches early; batch-1 KV load during pair 2
            prologue = {}
            for pair in range(1, PAIRS):
                if pair == HPC:
                    continue  # loaded by _load_kv(1)
                prologue.setdefault(max(0, first_idx[pair] - 6), []).append(
                    lambda p=pair: _load_qt(p)
                )
            prologue.setdefault(first_idx[2], []).append(lambda: _load_kv(1))

            # 2-deep software pipeline: in the window where ACT runs exp(k),
            # the PE executes QK(k+1) first (data ready at window start, so
            # exp(k+1) is never gated by QK latency) and then PV(k-1) (whose
            # pt has been ready since exp(k-1) ended - no sem-latency stall).
            n = len(sched)
            for i in range(n + 2):
                for fn in prologue.get(i, []):
                    fn()
                if i < n:
                    emit_qk(sched[i])
                if 1 <= i <= n:
                    emit_exp(sched[i - 1])
                if 2 <= i:
                    bb = sched[i - 2]
                    emit_pv(bb)
                    if bb.last_of_group:
                        emit_norm(bb)

    nc.compile()
    return nc


_NC = {}


def _get_nc(uniform_mask: bool = True):
    if uniform_mask not in _NC:
        _NC[uniform_mask] = build_module(uniform_mask)
    return _NC[uniform_mask]


def shard_inputs(q, kv, key_padding_mask):
    """Full inputs -> list of 8 per-core input maps."""
    import ml_dtypes

    bf16 = ml_dtypes.bfloat16
    q = np.asarray(q, dtype=np.float32)
    kv = np.asarray(kv, dtype=np.float32)
    mask = np.asarray(key_padding_mask)

    pbias = np.where(mask, np.float32(0.0), np.float32(NEG)).astype(np.float32)

    # tri01[k, qq] = 1 where k <= qq (keys on partitions)
    kk = np.arange(128)[:, None]
    qq = np.arange(128)[None, :]
    tri01 = (kk <= qq).astype(bf16)

    in_maps = []
    for c in range(N_CORES):
        qc = q[:, :, HPC * c : HPC * (c + 1), :]  # [B, S, 4, D]
        qtc = (
            (np.ascontiguousarray(np.transpose(qc, (0, 2, 3, 1)))
             * np.float32(A_SCH))
            .reshape(PAIRS, D, S)
            .astype(bf16)
        )
        kc = kv[:, :, 0, c, :]  # [B, S, D]
        vc = kv[:, :, 1, c, :]  # [B, S, D]
        ktc = np.ascontiguousarray(np.transpose(kc, (0, 2, 1))).astype(bf16)
        # vx[b, k, j, 0:128] = v[b, 128j+k, :]; [..., 128] = 1; pad 0
        vxc = np.zeros((B, D, NCHUNK, VW), dtype=bf16)
        vxc[:, :, :, :128] = np.transpose(
            vc.reshape(B, NCHUNK, 128, D), (0, 2, 1, 3)
        )
        vxc[:, :, :, 128] = bf16(1.0)
        in_maps.append(
            {"qt": qtc, "kt": ktc, "vx": vxc, "tri": tri01, "pb": pbias}
        )
    return in_maps


def unshard_output(results):
    """Per-core 'ot' [PAIRS, NG, 128, 4, 128] -> full [B, S, H, D] fp32."""
    out = np.empty((B, S, H, D), dtype=np.float32)
    for c in range(N_CORES):
        otc = np.asarray(results[c]["ot"], dtype=np.float32)
        for pair in range(PAIRS):
            b, h = pair // HPC, HPC * c + pair % HPC
            # [NG, 128p, 4t, D] -> [NG, 4t, 128p, D] -> [S, D]
            out[b, :, h, :] = np.transpose(otc[pair], (0, 2, 1, 3)).reshape(S, D)
    return out


def kernel(q, kv, key_padding_mask):
    uniform = bool(np.asarray(key_padding_mask).all())
    nc = _get_nc(uniform)
    in_maps = shard_inputs(q, kv, key_padding_mask)
    res = run_bass_kernel_spmd(nc, in_maps, core_ids=list(range(N_CORES)))
    return unshard_output(res.results)
